# revision 80
# baseline (speedup 1.0000x reference)
"""Trainium2 Bass kernel for nn_SpaceTimeAtten (space-time attention block).

Contract: kernel(**inputs) takes FULL unsharded numpy inputs (see reference
setup_inputs) and returns the FULL (2, 512, 8, 28, 28) float32 output.

Sharding: 8 cores = 2 batches x 4 query-chunks of 1664 t-positions. Each core:
  - runs the local convs (Q=ph_x, wy, pm) first to fill the startup DMA window;
    BN partial sums are folded into the wy bias-add (accum_out) plus Square
    activations on the otherwise-idle scalar engine,
  - Q/K/V projections and the energy matmul run in fp8e4 with DoubleRow perf
    mode (2 contraction tiles per instruction); P and ph_m stay bf16 (fp8's
    dynamic range cannot hold exp(E - global_max) across rows). Numerics
    verified end-to-end: rel err 2.5e-3 vs the 2e-2 tolerance, dominated by
    bf16 wy, not the attention path,
  - attention keeps the energy matrix TRANSPOSED (E^T = [s_part, t_free]) so
    exp(E^T - M1) is directly the lhsT operand of the PV matmul, and the PV is
    produced in [c, t] form (lhsT = ph_m tile) so the accumulator layout
    equals the output layout - no transposes anywhere,
  - row-sums r_t come from free-dim matmuls against a ones vector; 1/r is
    broadcast to 128 partitions by a 1-partition-lhsT matmul, with a +1e30
    additive mask folding invalid-t handling into the reciprocal,
  - the BN AllReduce is gated on attention block 0 so it executes in a
    DMA-quiet window (its rings otherwise starve the K/V piece stream); the
    second-softmax denominators are the only end-of-kernel collective, and
    everything not depending on it (BN scales, wy*alpha+beta) runs under it.
"""

import numpy as np

# ---- problem constants (hardcoded per contract) ----
N_B, C, T, H, W = 2, 512, 8, 28, 28
THW = T * H * W            # 6272
BN_EPS = 1e-5

CI = 4                     # input-channel 128-chunks
CO = 4                     # output-channel 128-chunks
S_PAD = 6272               # 49 s-tiles of 128 (exact, no padding)
NST = 49
T_LOC = 1664               # local t per core (13 tiles of 128)
NTT = 13
BLOCKS = [(0, 4), (4, 4), (8, 3), (11, 2)]   # (t-tile start, n tiles)

_PROG_CACHE = {}


def _build_program(m1, m2, gamma, debug=False):
    import concourse.bass as bass
    import concourse.mybir as mybir
    import concourse.tile as tile
    from concourse import bacc

    N_B, C = 2, 512
    THW = 6272
    BN_EPS = 1e-5
    CI = CO = 4
    S_PAD = 6272
    NST = 49
    T_LOC = 1664
    BLOCKS = [(0, 4), (4, 4), (8, 3), (11, 2)]

    f32 = mybir.dt.float32
    f32r = mybir.dt.float32r
    bf16 = mybir.dt.bfloat16
    fp8 = mybir.dt.float8e4
    DBLROW = mybir.MatmulPerfMode.DoubleRow
    EXP = mybir.ActivationFunctionType.Exp
    COPY_FN = mybir.ActivationFunctionType.Copy
    SQRT = mybir.ActivationFunctionType.Sqrt
    AX = mybir.AxisListType.X
    MUL = mybir.AluOpType.mult
    ADD = mybir.AluOpType.add

    nc = bacc.Bacc("TRN2")

    x8_full = nc.dram_tensor("x8_full", [C, S_PAD], fp8, kind="ExternalInput")
    mask8_full = nc.dram_tensor("mask8_full", [C, S_PAD], fp8, kind="ExternalInput")
    x_loc = nc.dram_tensor("x_loc", [C, T_LOC], f32, kind="ExternalInput")
    x8_loc = nc.dram_tensor("x8_loc", [C, T_LOC], fp8, kind="ExternalInput")
    wht8 = nc.dram_tensor("wht8", [C, C], fp8, kind="ExternalInput")
    wgt8 = nc.dram_tensor("wgt8", [C, C], fp8, kind="ExternalInput")
    wmt = nc.dram_tensor("wmt", [C, C], f32r, kind="ExternalInput")
    wzt = nc.dram_tensor("wzt", [C, C], f32, kind="ExternalInput")
    bh_in = nc.dram_tensor("bh_in", [128, CO], f32, kind="ExternalInput")
    bg_in = nc.dram_tensor("bg_in", [128, CO], f32, kind="ExternalInput")
    bm_in = nc.dram_tensor("bm_in", [128, CO], f32, kind="ExternalInput")
    bz_in = nc.dram_tensor("bz_in", [128, CO], f32, kind="ExternalInput")
    bh_row_in = nc.dram_tensor("bh_row_in", [128, C], f32, kind="ExternalInput")
    bnw_in = nc.dram_tensor("bnw_in", [128, CO], f32, kind="ExternalInput")
    bnb_in = nc.dram_tensor("bnb_in", [128, CO], f32, kind="ExternalInput")
    ones_in = nc.dram_tensor("ones_in", [128, 128], bf16, kind="ExternalInput")
    maskhuge_in = nc.dram_tensor("maskhuge_in", [C, T_LOC], f32, kind="ExternalInput")
    bzc_in = nc.dram_tensor("bzc_in", [128, 8], f32, kind="ExternalInput")
    bsel_in = nc.dram_tensor("bsel_in", [128, 2], f32, kind="ExternalInput")

    out_loc = nc.dram_tensor("out_loc", [C, T_LOC], f32, kind="ExternalOutput")
    if debug:
        d_phx = nc.dram_tensor("d_phx", [C, T_LOC], f32, kind="ExternalOutput")
        d_z = nc.dram_tensor("d_z", [C, T_LOC], f32, kind="ExternalOutput")
        d_wy = nc.dram_tensor("d_wy", [C, T_LOC], f32, kind="ExternalOutput")

    cc_bn_in = nc.dram_tensor("cc_bn_in", [128, 8], f32)
    cc_bn_out = nc.dram_tensor("cc_bn_out", [128, 8], f32)
    cc_se_in = nc.dram_tensor("cc_se_in", [128, 8], f32)
    cc_se_out = nc.dram_tensor("cc_se_out", [128, 8], f32)

    def dview(dram):
        return dram.rearrange("(k p) s -> p k s", p=128)

    FC = T_LOC // 4  # 416

    with tile.TileContext(nc) as tc:
        with (
            tc.tile_pool(name="const", bufs=1) as cpool,
            tc.tile_pool(name="ptile", bufs=8) as ptpool,
            tc.tile_pool(name="small", bufs=1) as spool,
        ):
            # ---- constants (gpsimd queue) ----
            ones_t = cpool.tile([128, 128], bf16, tag="ones")
            nc.gpsimd.dma_start(out=ones_t[:], in_=ones_in[:])
            bh_t = cpool.tile([128, CO], f32, tag="bh")
            bg_t = cpool.tile([128, CO], f32, tag="bg")
            bm_t = cpool.tile([128, CO], f32, tag="bm")
            bz_t = cpool.tile([128, CO], f32, tag="bz")
            bnw_t = cpool.tile([128, CO], f32, tag="bnw")
            bnb_t = cpool.tile([128, CO], f32, tag="bnb")
            for tl, dr in ((bh_t, bh_in), (bg_t, bg_in), (bm_t, bm_in),
                           (bz_t, bz_in), (bnw_t, bnw_in), (bnb_t, bnb_in)):
                nc.gpsimd.dma_start(out=tl[:], in_=dr[:])
            m2bh = cpool.tile([128, CO], f32, tag="m2bh")
            bsel_t = cpool.tile([128, 2], f32, tag="bsel")
            nc.gpsimd.dma_start(out=bsel_t[:], in_=bsel_in[:])
            maskhuge = cpool.tile([128, T_LOC], f32, tag="maskhuge")
            nc.gpsimd.dma_start(out=maskhuge[:], in_=dview(maskhuge_in)[:, 0, :])
            bzc_t = cpool.tile([128, 8], f32, tag="bzc")
            nc.gpsimd.dma_start(out=bzc_t[:], in_=bzc_in[:])
            m1b = cpool.tile([128, 1], f32, tag="m1b")
            nc.vector.memset(m1b[:], -m1)
            m2b = cpool.tile([128, 1], f32, tag="m2b")
            nc.vector.memset(m2b[:], -m2)
            nc.vector.tensor_scalar_add(m2bh[:], bh_t[:], -m2)

            # long-lived SBUF tensors (left-stack bottom: released last)
            p_phx = tc.alloc_tile_pool(name="phxp", bufs=1)
            phx = p_phx.tile([128, CI, T_LOC], fp8, tag="phx")
            p_res = tc.alloc_tile_pool(name="resp", bufs=1, side="right")
            pm_t = p_res.tile([128, CO, T_LOC], bf16, tag="pm")
            wy_t = p_res.tile([128, CO, T_LOC], bf16, tag="wy")

            # ---- weights + x_loc (sync queue; first-needed first) ----
            p_w1 = tc.alloc_tile_pool(name="w1", bufs=1)
            wt_h8 = p_w1.tile([128, CI, C], fp8, tag="wh8")
            wt_g8 = p_w1.tile([128, CI, C], fp8, tag="wg8")
            p_w2 = tc.alloc_tile_pool(name="w2", bufs=1)
            wt_z = p_w2.tile([128, CI, C], f32r, tag="wz")
            wt_m = p_w2.tile([128, CI, C], f32r, tag="wm")
            p_xl = tc.alloc_tile_pool(name="xlp", bufs=1)
            xloc_t = p_xl.tile([128, CI, T_LOC], f32r, tag="xloc")
            xloc8_t = p_xl.tile([128, CI, T_LOC], fp8, tag="xloc8")
            nc.sync.dma_start(out=wt_h8[:], in_=dview(wht8))
            nc.sync.dma_start(out=xloc8_t[:], in_=dview(x8_loc))
            nc.sync.dma_start(out=wt_z[:], in_=dview(wzt).bitcast(f32r))
            for fc in range(4):
                nc.sync.dma_start(
                    out=xloc_t[:, :, fc * FC:(fc + 1) * FC],
                    in_=dview(x_loc).bitcast(f32r)[:, :, fc * FC:(fc + 1) * FC])
            nc.sync.dma_start(out=wt_m[:], in_=dview(wmt))
            nc.sync.dma_start(out=wt_g8[:], in_=dview(wgt8))

            stats_bn = spool.tile([128, 8], f32, tag="statsbn")
            se_tot = spool.tile([128, CO], f32, tag="setot")

            # ======== P0: local convs (Q, wy, pm) + BN partials ========
            ps_c = tc.alloc_tile_pool(name="psc", bufs=2, space="PSUM")
            p_scr = tc.alloc_tile_pool(name="scrp", bufs=2)
            for fc in range(4):
                for co in range(CO):
                    ps = ps_c.tile([128, 512], f32, tag="c")
                    for k in range(2):
                        nc.tensor.matmul(
                            ps[:, :FC],
                            wt_h8[:, 2 * k:2 * k + 2, co * 128:(co + 1) * 128],
                            xloc8_t[:, 2 * k:2 * k + 2, fc * FC:(fc + 1) * FC],
                            start=(k == 0), stop=(k == 1),
                            perf_mode=DBLROW)
                    nc.vector.tensor_scalar_add(
                        phx[:, co, fc * FC:(fc + 1) * FC],
                        ps[:, :FC], bh_t[:, co:co + 1])
            # wy conv; BN sum folded into the bias-add via accum_out, square
            # sums on the otherwise-idle scalar engine
            SQUARE = mybir.ActivationFunctionType.Square
            sbn_p = spool.tile([128, 16], f32, tag="sbnp")
            for fc in range(4):
                for co in range(CO):
                    ps = ps_c.tile([128, 512], f32, tag="c")
                    for ci in range(CI):
                        nc.tensor.matmul(
                            ps[:, :FC],
                            wt_z[:, ci, co * 128:(co + 1) * 128],
                            xloc_t[:, ci, fc * FC:(fc + 1) * FC],
                            start=(ci == 0), stop=(ci == CI - 1))
                    nc.vector.tensor_scalar(
                        wy_t[:, co, fc * FC:(fc + 1) * FC],
                        ps[:, :FC], bz_t[:, co:co + 1], 0.0, op0=ADD, op1=ADD,
                        accum_out=sbn_p[:, fc * 4 + co:fc * 4 + co + 1])
            nc.vector.tensor_add(stats_bn[:, 0:4], sbn_p[:, 0:4], sbn_p[:, 4:8])
            nc.vector.tensor_add(stats_bn[:, 0:4], stats_bn[:, 0:4],
                                 sbn_p[:, 8:12])
            nc.vector.tensor_add(stats_bn[:, 0:4], stats_bn[:, 0:4],
                                 sbn_p[:, 12:16])
            for co in range(CO):
                scr = p_scr.tile([128, T_LOC], bf16, tag="scr")
                nc.scalar.activation(scr[:], wy_t[:, co, :], SQUARE,
                                     accum_out=stats_bn[:, 4 + co:5 + co])
            for fc in range(4):
                for co in range(CO):
                    ps = ps_c.tile([128, 512], f32, tag="c")
                    for ci in range(CI):
                        nc.tensor.matmul(
                            ps[:, :FC],
                            wt_m[:, ci, co * 128:(co + 1) * 128],
                            xloc_t[:, ci, fc * FC:(fc + 1) * FC],
                            start=(ci == 0), stop=(ci == CI - 1))
                    nc.vector.tensor_scalar_add(
                        pm_t[:, co, fc * FC:(fc + 1) * FC],
                        ps[:, :FC], bm_t[:, co:co + 1])
            p_scr.release()
            if debug:
                dwy_p = tc.alloc_tile_pool(name="dwyp", bufs=1)
                dwy_f = dwy_p.tile([128, CO, T_LOC], f32, tag="dwy")
                for co in range(CO):
                    nc.vector.tensor_copy(dwy_f[:, co, :], wy_t[:, co, :])
                nc.sync.dma_start(out=dview(d_wy), in_=dwy_f[:])
                dwy_p.release()
            p_xl.release()
            p_w2.release()

            # ======== P1: K/V convs over full s-range (bf16 outputs) ========
            p_kv = tc.alloc_tile_pool(name="kvp", bufs=1, side="right")
            pgh = p_kv.tile([128, CI, S_PAD], fp8, tag="pgh")
            phmh = p_kv.tile([128, NST, C], bf16, tag="phmh")
            p_piece = tc.alloc_tile_pool(name="piecep", bufs=2)

            pieces = []
            o = 0
            while o < NST:
                w = min(4, NST - o)
                pieces.append((o, w))
                o += w
            for (pt0, ptw) in pieces:
                s_off = pt0 * 128
                pw = ptw * 128
                xp = p_piece.tile([128, CI, 512], fp8, tag="xp", bufs=3,
                                  name="xp")
                nc.sync.dma_start(
                    out=xp[:, :, :pw],
                    in_=dview(x8_full)[:, :, s_off:s_off + pw])
                for co in range(CO):
                    ps = ps_c.tile([128, 512], f32, tag="c")
                    for k in range(2):
                        nc.tensor.matmul(
                            ps[:, :pw],
                            wt_g8[:, 2 * k:2 * k + 2, co * 128:(co + 1) * 128],
                            xp[:, 2 * k:2 * k + 2, :pw],
                            start=(k == 0), stop=(k == 1),
                            perf_mode=DBLROW)
                    nc.vector.tensor_scalar_add(
                        pgh[:, co, s_off:s_off + pw],
                        ps[:, :pw], bg_t[:, co:co + 1])
                mp = p_piece.tile([128, CI, 512], fp8, tag="mp", bufs=3,
                                  name="mp")
                nc.gpsimd.dma_start(
                    out=mp[:, :, :pw],
                    in_=dview(mask8_full)[:, :, s_off:s_off + pw])
                for sj in range(ptw):
                    st = pt0 + sj
                    ps = ps_c.tile([128, 512], f32, tag="c")
                    for k in range(2):
                        nc.tensor.matmul(
                            ps[:],
                            mp[:, 2 * k:2 * k + 2, sj * 128:(sj + 1) * 128],
                            wt_h8[:, 2 * k:2 * k + 2, :],
                            start=(k == 0), stop=(k == 1),
                            perf_mode=DBLROW)
                    nc.scalar.activation(phmh[:, st, :], ps[:], COPY_FN)

            if debug:
                p_dbg = tc.alloc_tile_pool(name="dbgp", bufs=1)
                dphx_f = p_dbg.tile([128, CI, T_LOC], f32, tag="dphx")
                for ci in range(CI):
                    nc.vector.tensor_copy(dphx_f[:, ci, :], phx[:, ci, :])
                nc.sync.dma_start(out=dview(d_phx), in_=dphx_f[:])
                p_dbg.release()
            ps_c.release()
            p_piece.release()
            p_w1.release()

            # ======== P2: attention, single pass, [c,t]-form PV ========
            # PSUM: o x4 (c-chunks) + e x3 (rb shares the e tag) + r = 8 banks
            ps_att = tc.alloc_tile_pool(name="psa", bufs=1, space="PSUM")
            p_z = tc.alloc_tile_pool(name="zp", bufs=2)
            for bi, (t0, nt) in enumerate(BLOCKS):
                tfree = nt * 128
                trange = slice(t0 * 128, t0 * 128 + tfree)
                ops = [ps_att.tile([128, 512], f32, tag=f"o{j}", name=f"o{j}")
                       for j in range(CO)]
                rps = ps_att.tile([128, 512], f32, tag="r", name="rps")
                for st in range(NST):
                    eps_t = ps_att.tile([128, 512], f32, tag="e", bufs=3,
                                        name="eps")
                    for k in range(2):
                        nc.tensor.matmul(
                            eps_t[:, :tfree],
                            pgh[:, 2 * k:2 * k + 2, st * 128:(st + 1) * 128],
                            phx[:, 2 * k:2 * k + 2, trange],
                            start=(k == 0), stop=(k == 1),
                            perf_mode=DBLROW)
                    pt = ptpool.tile([128, 512], bf16, tag="pt")
                    nc.scalar.activation(pt[:, :tfree], eps_t[:, :tfree],
                                         EXP, bias=m1b[:], scale=1.0)
                    for co in range(CO):
                        nc.tensor.matmul(
                            ops[co][:, :tfree],
                            phmh[:, st, co * 128:(co + 1) * 128],
                            pt[:, :tfree],
                            start=(st == 0), stop=(st == NST - 1))
                    nc.tensor.matmul(
                        rps[:, :tfree],
                        ones_t[:],
                        pt[:, :tfree],
                        start=(st == 0), stop=(st == NST - 1))

                # block tail: rps already holds r on every partition, so
                # 1/r is a direct full-width masked add + reciprocal
                rba = p_z.tile([128, 512], f32, tag="rba")
                nc.vector.tensor_add(rba[:, :tfree], rps[:, :tfree],
                                     maskhuge[:, trange])
                rb = p_z.tile([128, 512], f32, tag="rb")
                nc.vector.reciprocal(rb[:, :tfree], rba[:, :tfree])
                se_blk = spool.tile([128, CO], f32, tag=f"seblk{bi}")
                for co in range(CO):
                    zt = p_z.tile([128, 512], f32, tag="z")
                    nc.vector.tensor_mul(zt[:, :tfree], ops[co][:, :tfree],
                                         rb[:, :tfree])
                    ez = p_z.tile([128, 512], bf16, tag="ez")
                    nc.scalar.activation(ez[:, :tfree], zt[:, :tfree],
                                         EXP, bias=m2bh[:, co:co + 1], scale=1.0,
                                         accum_out=se_blk[:, co:co + 1])
                    nc.vector.tensor_mul(pm_t[:, co, trange], ez[:, :tfree],
                                         pm_t[:, co, trange])
                    if debug:
                        nc.sync.dma_start(out=dview(d_z)[:, co, trange],
                                          in_=zt[:, :tfree])
                if bi == 0:
                    nc.vector.tensor_copy(se_tot[:], se_blk[:])
                    # BN collective gated on block-0 completion: it then runs
                    # in a DMA-quiet window instead of starving the piece
                    # stream (its rings monopolize the DMA engines). The
                    # rewrite of stats_bn[:, 0:1] below adds the dependency.
                    one_dep = spool.tile([128, 1], f32, tag="onedep")
                    nc.vector.tensor_scalar(one_dep[:], se_blk[:, 0:1],
                                            0.0, 1.0, op0=MUL, op1=ADD)
                    nc.vector.tensor_scalar_mul(stats_bn[:, 0:1],
                                                stats_bn[:, 0:1], one_dep[:])
                    nc.gpsimd.dma_start(out=cc_bn_in[:], in_=stats_bn[:])
                    nc.gpsimd.collective_compute(
                        "AllReduce", mybir.AluOpType.add,
                        replica_groups=[[0, 1, 2, 3, 4, 5, 6, 7]],
                        ins=[cc_bn_in[:]], outs=[cc_bn_out[:]])
                else:
                    nc.vector.tensor_add(se_tot[:], se_tot[:], se_blk[:])
            ps_att.release()
            p_z.release()
            p_kv.release()
            p_phx.release()

            # ======== P3: SE collective + finale ========
            stats_se = spool.tile([128, 8], f32, tag="statsse")
            nc.vector.tensor_scalar_mul(stats_se[:, 0:CO], se_tot[:],
                                        bsel_t[:, 0:1])
            nc.vector.tensor_scalar_mul(stats_se[:, CO:2 * CO], se_tot[:],
                                        bsel_t[:, 1:2])
            nc.sync.dma_start(out=cc_se_in[:], in_=stats_se[:])
            nc.gpsimd.collective_compute(
                "AllReduce", mybir.AluOpType.add,
                replica_groups=[[0, 1, 2, 3, 4, 5, 6, 7]],
                ins=[cc_se_in[:]], outs=[cc_se_out[:]])

            # BN finale scales: depend only on the EARLY collective, so they
            # run under the SE collective
            p_ot = tc.alloc_tile_pool(name="otp", bufs=1)
            ot_t = p_ot.tile([128, CO, T_LOC], f32, tag="ot")
            gbn = spool.tile([128, 8], f32, tag="gbn")
            nc.gpsimd.dma_start(out=gbn[:], in_=cc_bn_out[:])
            cnt = 1.0 / (N_B * THW)
            mu = spool.tile([128, CO], f32, tag="mu")
            nc.vector.tensor_scalar_mul(mu[:], gbn[:, 0:CO], cnt)
            nc.vector.tensor_sub(mu[:], mu[:], bzc_t[:, 0:CO])
            ex2 = spool.tile([128, CO], f32, tag="ex2")
            nc.vector.tensor_scalar_mul(ex2[:], gbn[:, 4:4 + CO], cnt)
            nc.vector.tensor_sub(ex2[:], ex2[:], bzc_t[:, CO:2 * CO])
            var = spool.tile([128, CO], f32, tag="var")
            nc.vector.tensor_mul(var[:], mu[:], mu[:])
            nc.vector.tensor_sub(var[:], ex2[:], var[:])
            nc.vector.tensor_scalar_add(var[:], var[:], BN_EPS)
            std = spool.tile([128, CO], f32, tag="std")
            nc.scalar.activation(std[:], var[:], SQRT)
            nc.vector.reciprocal(std[:], std[:])
            alpha = spool.tile([128, CO], f32, tag="alpha")
            nc.vector.tensor_mul(alpha[:], std[:], bnw_t[:])
            beta = spool.tile([128, CO], f32, tag="beta")
            nc.vector.tensor_mul(beta[:], mu[:], alpha[:])
            nc.vector.tensor_sub(beta[:], bnb_t[:], beta[:])
            for co in range(CO):
                nc.vector.tensor_scalar(ot_t[:, co, :], wy_t[:, co, :],
                                        alpha[:, co:co + 1], beta[:, co:co + 1],
                                        op0=MUL, op1=ADD)

            gst = spool.tile([128, 8], f32, tag="gst")
            nc.sync.dma_start(out=gst[:], in_=cc_se_out[:])
            gse = spool.tile([128, CO], f32, tag="gse")
            tmp_a = spool.tile([128, CO], f32, tag="tmpa")
            nc.vector.tensor_scalar_mul(gse[:], gst[:, 0:CO], bsel_t[:, 0:1])
            nc.vector.tensor_scalar_mul(tmp_a[:], gst[:, CO:2 * CO],
                                        bsel_t[:, 1:2])
            nc.vector.tensor_add(gse[:], gse[:], tmp_a[:])
            nc.vector.reciprocal(gse[:], gse[:])
            nc.vector.tensor_scalar_mul(gse[:], gse[:], gamma)

            p_out = tc.alloc_tile_pool(name="outp", bufs=4)
            for co in range(CO):
                mt = p_out.tile([128, T_LOC], f32, tag="mt")
                nc.scalar.activation(mt[:], pm_t[:, co, :], COPY_FN,
                                     scale=gse[:, co:co + 1])
                nc.vector.tensor_add(mt[:], mt[:], ot_t[:, co, :])
                nc.sync.dma_start(out=dview(out_loc)[:, co, :], in_=mt[:])
            p_out.release()
            p_ot.release()
            p_res.release()

    nc.compile()
    return nc


def _prepare_maps(x, mask, Wh, bh, Wg, bg, Wm, bm, Wz, bz, bn_w, bn_b):
    import ml_dtypes

    xf = np.ascontiguousarray(x.reshape(N_B, C, THW), dtype=np.float32)
    mf = np.ascontiguousarray(mask.reshape(N_B, C, THW), dtype=np.float32)

    def chunked_bias(b):
        return np.ascontiguousarray(b.reshape(CO, 128).T, dtype=np.float32)

    wht = np.ascontiguousarray(Wh.T, dtype=np.float32)
    wgt = np.ascontiguousarray(Wg.T, dtype=np.float32)
    wmt = np.ascontiguousarray(Wm.T, dtype=np.float32)
    wzt = np.ascontiguousarray(Wz.T, dtype=np.float32)
    wht8 = wht.astype(ml_dtypes.float8_e4m3)
    wgt8 = wgt.astype(ml_dtypes.float8_e4m3)
    bh_row = np.broadcast_to(bh.astype(np.float32), (128, C)).copy()

    # BN bias compensation: raw sums include (8*T_LOC - N*THW) padded columns
    # where wy == bz exactly (x padded with zeros).
    n_pad = 8 * T_LOC - N_B * THW
    cntf = 1.0 / (N_B * THW)
    bzc = np.zeros((128, 8), np.float32)
    bzc[:, 0:4] = chunked_bias(bz * (n_pad * cntf))
    bzc[:, 4:8] = chunked_bias((bz * bz) * (n_pad * cntf))

    in_maps = []
    for core in range(8):
        n, q = divmod(core, 4)
        t0 = T_LOC * q
        valid = int(np.clip(THW - t0, 0, T_LOC))
        x_locc = np.zeros((C, T_LOC), np.float32)
        x_locc[:, :valid] = xf[n][:, t0:t0 + valid]
        x8_locc = x_locc.astype(ml_dtypes.float8_e4m3)
        # additive +huge mask on r for invalid t: 1/(r+1e30) ~ 0 => z ~ 0
        # => expz ~ exp(-M2) ~ 0
        mrow = np.full((T_LOC,), 1e-30, np.float32)
        mrow[valid:] = 1e30
        maskhuge = np.broadcast_to(mrow, (C, T_LOC)).copy()
        bsel = np.zeros((128, 2), np.float32)
        bsel[:, 0] = 1.0 if n == 0 else 0.0
        bsel[:, 1] = 0.0 if n == 0 else 1.0
        in_maps.append(dict(
            x8_full=xf[n].astype(ml_dtypes.float8_e4m3),
            mask8_full=mf[n].astype(ml_dtypes.float8_e4m3),
            x_loc=x_locc, x8_loc=x8_locc, wht8=wht8, wgt8=wgt8,
            wmt=wmt, wzt=wzt,
            bh_in=chunked_bias(bh), bg_in=chunked_bias(bg),
            bm_in=chunked_bias(bm), bz_in=chunked_bias(bz),
            bh_row_in=bh_row,
            bnw_in=chunked_bias(bn_w), bnb_in=chunked_bias(bn_b),
            ones_in=np.ones((128, 128), dtype=ml_dtypes.bfloat16),
            ones_row_in=np.ones((1, 128), np.float32),
            maskhuge_in=maskhuge, bzc_in=bzc,
            bsel_in=bsel,
        ))
    return in_maps


def _estimate_shifts(xf, mf, Wh, bh, Wg, bg):
    # M1: safe global upper-bound estimate for the max of the energy matrix.
    # Any M1 in [true_max - 80, min_row_max + 85] keeps softmax exact
    # (constant shifts cancel); the window is tens wide so a sampled
    # estimate plus margin is bulletproof.
    ti = np.arange(0, THW, 41)
    si = np.arange(0, THW, 7)
    m_s = -np.inf
    for n in range(N_B):
        Q = (Wh @ xf[n][:, ti]) + bh[:, None]
        K = (Wg @ xf[n][:, si]) + bg[:, None]
        m_s = max(m_s, float((Q.T @ K).max()))
    m1 = m_s + 5.0
    # M2: norm bound on |ph_m| entries (second softmax argument is a convex
    # combination of ph_m values, so bounded by max |ph_m|).
    whn = float(np.linalg.norm(Wh, axis=1).max())
    mcn = max(float(np.linalg.norm(mf[n], axis=0).max()) for n in range(N_B))
    m2 = whn * mcn + float(np.abs(bh).max()) + 1.0
    return m1, m2


def kernel(x, mask, Wh, bh, Wg, bg, Wm, bm, Wz, bz, bn_w, bn_b, gamma,
           _debug=False, _trace=False):
    from concourse.bass_utils import run_bass_kernel_spmd

    x = np.asarray(x, np.float32)
    mask = np.asarray(mask, np.float32)
    Wh = np.asarray(Wh, np.float32); bh = np.asarray(bh, np.float32)
    Wg = np.asarray(Wg, np.float32); bg = np.asarray(bg, np.float32)
    Wm = np.asarray(Wm, np.float32); bm = np.asarray(bm, np.float32)
    Wz = np.asarray(Wz, np.float32); bz = np.asarray(bz, np.float32)
    bn_w = np.asarray(bn_w, np.float32); bn_b = np.asarray(bn_b, np.float32)
    gammaf = float(np.asarray(gamma))

    xf = x.reshape(N_B, C, THW)
    mf = mask.reshape(N_B, C, THW)
    m1, m2 = _estimate_shifts(xf, mf, Wh, bh, Wg, bg)
    key = (round(m1, 1), round(m2, 1), round(gammaf, 6), bool(_debug))
    if key not in _PROG_CACHE:
        _PROG_CACHE[key] = _build_program(key[0], key[1], gammaf, debug=_debug)
    nc = _PROG_CACHE[key]

    in_maps = _prepare_maps(x, mask, Wh, bh, Wg, bg, Wm, bm, Wz, bz, bn_w, bn_b)
    res = run_bass_kernel_spmd(nc, in_maps, core_ids=list(range(8)), trace=_trace)

    out = np.empty((N_B, C, THW), np.float32)
    for core in range(8):
        n, q = divmod(core, 4)
        t0 = T_LOC * q
        valid = int(np.clip(THW - t0, 0, T_LOC))
        if valid > 0:
            out[n][:, t0:t0 + valid] = res.results[core]["out_loc"][:, :valid]
    out = out.reshape(N_B, C, T, H, W)
    if _debug or _trace:
        return out, res
    return out


# revision 82
# speedup vs baseline: 1.0248x; 1.0248x over previous
"""Trainium2 Bass kernel for nn_SpaceTimeAtten (space-time attention block).

Contract: kernel(**inputs) takes FULL unsharded numpy inputs (see reference
setup_inputs) and returns the FULL (2, 512, 8, 28, 28) float32 output.

Sharding: 8 cores = 2 batches x 4 query-chunks of 1664 t-positions. Each core:
  - runs the local convs (Q=ph_x, wy, pm) first to fill the startup DMA window;
    BN partial sums are folded into the wy bias-add (accum_out) plus Square
    activations on the otherwise-idle scalar engine,
  - Q/K/V projections and the energy matmul run in fp8e4 with DoubleRow perf
    mode (2 contraction tiles per instruction); P and ph_m stay bf16 (fp8's
    dynamic range cannot hold exp(E - global_max) across rows). Numerics
    verified end-to-end: rel err 2.5e-3 vs the 2e-2 tolerance, dominated by
    bf16 wy, not the attention path,
  - attention keeps the energy matrix TRANSPOSED (E^T = [s_part, t_free]) so
    exp(E^T - M1) is directly the lhsT operand of the PV matmul, and the PV is
    produced in [c, t] form (lhsT = ph_m tile) so the accumulator layout
    equals the output layout - no transposes anywhere,
  - row-sums r_t come from free-dim matmuls against a ones vector; 1/r is
    broadcast to 128 partitions by a 1-partition-lhsT matmul, with a +1e30
    additive mask folding invalid-t handling into the reciprocal,
  - the BN AllReduce is gated on attention block 0 so it executes in a
    DMA-quiet window (its rings otherwise starve the K/V piece stream); the
    second-softmax denominators are the only end-of-kernel collective, and
    everything not depending on it (BN scales, wy*alpha+beta) runs under it.
"""

import numpy as np

# ---- problem constants (hardcoded per contract) ----
N_B, C, T, H, W = 2, 512, 8, 28, 28
THW = T * H * W            # 6272
BN_EPS = 1e-5

CI = 4                     # input-channel 128-chunks
CO = 4                     # output-channel 128-chunks
S_PAD = 6272               # 49 s-tiles of 128 (exact, no padding)
NST = 49
T_LOC = 1664               # local t per core (13 tiles of 128)
NTT = 13
BLOCKS = [(0, 4), (4, 4), (8, 3), (11, 2)]   # (t-tile start, n tiles)

_PROG_CACHE = {}


def _build_program(m1, m2, gamma, debug=False):
    import concourse.bass as bass
    import concourse.mybir as mybir
    import concourse.tile as tile
    from concourse import bacc

    N_B, C = 2, 512
    THW = 6272
    BN_EPS = 1e-5
    CI = CO = 4
    S_PAD = 6272
    NST = 49
    T_LOC = 1664
    BLOCKS = [(0, 4), (4, 4), (8, 3), (11, 2)]

    f32 = mybir.dt.float32
    f32r = mybir.dt.float32r
    bf16 = mybir.dt.bfloat16
    fp8 = mybir.dt.float8e4
    DBLROW = mybir.MatmulPerfMode.DoubleRow
    EXP = mybir.ActivationFunctionType.Exp
    COPY_FN = mybir.ActivationFunctionType.Copy
    SQRT = mybir.ActivationFunctionType.Sqrt
    AX = mybir.AxisListType.X
    MUL = mybir.AluOpType.mult
    ADD = mybir.AluOpType.add

    nc = bacc.Bacc("TRN2")

    x8_full = nc.dram_tensor("x8_full", [C, S_PAD], fp8, kind="ExternalInput")
    mask8_full = nc.dram_tensor("mask8_full", [C, S_PAD], fp8, kind="ExternalInput")
    x_loc = nc.dram_tensor("x_loc", [C, T_LOC], f32, kind="ExternalInput")
    x8_loc = nc.dram_tensor("x8_loc", [C, T_LOC], fp8, kind="ExternalInput")
    wht8 = nc.dram_tensor("wht8", [C, C], fp8, kind="ExternalInput")
    wgt8 = nc.dram_tensor("wgt8", [C, C], fp8, kind="ExternalInput")
    wmt = nc.dram_tensor("wmt", [C, C], f32r, kind="ExternalInput")
    wzt = nc.dram_tensor("wzt", [C, C], f32, kind="ExternalInput")
    bh_in = nc.dram_tensor("bh_in", [128, CO], f32, kind="ExternalInput")
    bg_in = nc.dram_tensor("bg_in", [128, CO], f32, kind="ExternalInput")
    bm_in = nc.dram_tensor("bm_in", [128, CO], f32, kind="ExternalInput")
    bz_in = nc.dram_tensor("bz_in", [128, CO], f32, kind="ExternalInput")
    bh_row_in = nc.dram_tensor("bh_row_in", [128, C], f32, kind="ExternalInput")
    bnw_in = nc.dram_tensor("bnw_in", [128, CO], f32, kind="ExternalInput")
    bnb_in = nc.dram_tensor("bnb_in", [128, CO], f32, kind="ExternalInput")
    ones_in = nc.dram_tensor("ones_in", [128, 128], bf16, kind="ExternalInput")
    maskhuge_in = nc.dram_tensor("maskhuge_in", [1, T_LOC], f32, kind="ExternalInput")
    ones_row_in = nc.dram_tensor("ones_row_in", [1, 128], f32r, kind="ExternalInput")
    bzc_in = nc.dram_tensor("bzc_in", [128, 8], f32, kind="ExternalInput")
    bsel_in = nc.dram_tensor("bsel_in", [128, 2], f32, kind="ExternalInput")

    out_loc = nc.dram_tensor("out_loc", [C, T_LOC], f32, kind="ExternalOutput")
    if debug:
        d_phx = nc.dram_tensor("d_phx", [C, T_LOC], f32, kind="ExternalOutput")
        d_z = nc.dram_tensor("d_z", [C, T_LOC], f32, kind="ExternalOutput")
        d_wy = nc.dram_tensor("d_wy", [C, T_LOC], f32, kind="ExternalOutput")

    cc_bn_in = nc.dram_tensor("cc_bn_in", [128, 8], f32)
    cc_bn_out = nc.dram_tensor("cc_bn_out", [128, 8], f32)
    cc_se_in = nc.dram_tensor("cc_se_in", [128, 8], f32)
    cc_se_out = nc.dram_tensor("cc_se_out", [128, 8], f32)

    def dview(dram):
        return dram.rearrange("(k p) s -> p k s", p=128)

    FC = T_LOC // 4  # 416

    with tile.TileContext(nc) as tc:
        with (
            tc.tile_pool(name="const", bufs=1) as cpool,
            tc.tile_pool(name="ptile", bufs=8) as ptpool,
            tc.tile_pool(name="small", bufs=1) as spool,
        ):
            # ---- constants (gpsimd queue) ----
            ones_t = cpool.tile([128, 128], bf16, tag="ones")
            nc.gpsimd.dma_start(out=ones_t[:], in_=ones_in[:])
            bh_t = cpool.tile([128, CO], f32, tag="bh")
            bg_t = cpool.tile([128, CO], f32, tag="bg")
            bm_t = cpool.tile([128, CO], f32, tag="bm")
            bz_t = cpool.tile([128, CO], f32, tag="bz")
            bnw_t = cpool.tile([128, CO], f32, tag="bnw")
            bnb_t = cpool.tile([128, CO], f32, tag="bnb")
            for tl, dr in ((bh_t, bh_in), (bg_t, bg_in), (bm_t, bm_in),
                           (bz_t, bz_in), (bnw_t, bnw_in), (bnb_t, bnb_in)):
                nc.gpsimd.dma_start(out=tl[:], in_=dr[:])
            m2bh = cpool.tile([128, CO], f32, tag="m2bh")
            bsel_t = cpool.tile([128, 2], f32, tag="bsel")
            nc.gpsimd.dma_start(out=bsel_t[:], in_=bsel_in[:])
            maskhuge = cpool.tile([1, T_LOC], f32, tag="maskhuge")
            nc.gpsimd.dma_start(out=maskhuge[:], in_=maskhuge_in[:])
            ones_row = cpool.tile([1, 128], f32r, tag="onesrow")
            nc.gpsimd.dma_start(out=ones_row[:], in_=ones_row_in[:])
            bzc_t = cpool.tile([128, 8], f32, tag="bzc")
            nc.gpsimd.dma_start(out=bzc_t[:], in_=bzc_in[:])
            m1b = cpool.tile([128, 1], f32, tag="m1b")
            nc.vector.memset(m1b[:], -m1)
            m2b = cpool.tile([128, 1], f32, tag="m2b")
            nc.vector.memset(m2b[:], -m2)
            nc.vector.tensor_scalar_add(m2bh[:], bh_t[:], -m2)

            # long-lived SBUF tensors (left-stack bottom: released last)
            p_phx = tc.alloc_tile_pool(name="phxp", bufs=1)
            phx = p_phx.tile([128, CI, T_LOC], fp8, tag="phx")
            p_res = tc.alloc_tile_pool(name="resp", bufs=1, side="right")
            pm_t = p_res.tile([128, CO, T_LOC], bf16, tag="pm")
            wy_t = p_res.tile([128, CO, T_LOC], bf16, tag="wy")
            racc_row = p_res.tile([1, T_LOC], f32r, tag="racc")

            # ---- weights + x_loc (sync queue; first-needed first) ----
            p_w1 = tc.alloc_tile_pool(name="w1", bufs=1)
            wt_h8 = p_w1.tile([128, CI, C], fp8, tag="wh8")
            wt_g8 = p_w1.tile([128, CI, C], fp8, tag="wg8")
            p_w2 = tc.alloc_tile_pool(name="w2", bufs=1)
            wt_z = p_w2.tile([128, CI, C], f32r, tag="wz")
            wt_m = p_w2.tile([128, CI, C], f32r, tag="wm")
            p_xl = tc.alloc_tile_pool(name="xlp", bufs=1)
            xloc_t = p_xl.tile([128, CI, T_LOC], f32r, tag="xloc")
            xloc8_t = p_xl.tile([128, CI, T_LOC], fp8, tag="xloc8")
            nc.sync.dma_start(out=wt_h8[:], in_=dview(wht8))
            nc.sync.dma_start(out=xloc8_t[:], in_=dview(x8_loc))
            nc.sync.dma_start(out=wt_z[:], in_=dview(wzt).bitcast(f32r))
            for fc in range(4):
                nc.sync.dma_start(
                    out=xloc_t[:, :, fc * FC:(fc + 1) * FC],
                    in_=dview(x_loc).bitcast(f32r)[:, :, fc * FC:(fc + 1) * FC])
            nc.sync.dma_start(out=wt_m[:], in_=dview(wmt))
            nc.sync.dma_start(out=wt_g8[:], in_=dview(wgt8))

            stats_bn = spool.tile([128, 8], f32, tag="statsbn")
            se_tot = spool.tile([128, CO], f32, tag="setot")

            # ======== P0: local convs (Q, wy, pm) + BN partials ========
            ps_c = tc.alloc_tile_pool(name="psc", bufs=2, space="PSUM")
            p_scr = tc.alloc_tile_pool(name="scrp", bufs=2)
            for fc in range(4):
                for co in range(CO):
                    ps = ps_c.tile([128, 512], f32, tag="c")
                    for k in range(2):
                        nc.tensor.matmul(
                            ps[:, :FC],
                            wt_h8[:, 2 * k:2 * k + 2, co * 128:(co + 1) * 128],
                            xloc8_t[:, 2 * k:2 * k + 2, fc * FC:(fc + 1) * FC],
                            start=(k == 0), stop=(k == 1),
                            perf_mode=DBLROW)
                    nc.vector.tensor_scalar_add(
                        phx[:, co, fc * FC:(fc + 1) * FC],
                        ps[:, :FC], bh_t[:, co:co + 1])
            # wy conv; BN sum folded into the bias-add via accum_out, square
            # sums on the otherwise-idle scalar engine
            SQUARE = mybir.ActivationFunctionType.Square
            sbn_p = spool.tile([128, 16], f32, tag="sbnp")
            for fc in range(4):
                for co in range(CO):
                    ps = ps_c.tile([128, 512], f32, tag="c")
                    for ci in range(CI):
                        nc.tensor.matmul(
                            ps[:, :FC],
                            wt_z[:, ci, co * 128:(co + 1) * 128],
                            xloc_t[:, ci, fc * FC:(fc + 1) * FC],
                            start=(ci == 0), stop=(ci == CI - 1))
                    nc.vector.tensor_scalar(
                        wy_t[:, co, fc * FC:(fc + 1) * FC],
                        ps[:, :FC], bz_t[:, co:co + 1], 0.0, op0=ADD, op1=ADD,
                        accum_out=sbn_p[:, fc * 4 + co:fc * 4 + co + 1])
            nc.vector.tensor_add(stats_bn[:, 0:4], sbn_p[:, 0:4], sbn_p[:, 4:8])
            nc.vector.tensor_add(stats_bn[:, 0:4], stats_bn[:, 0:4],
                                 sbn_p[:, 8:12])
            nc.vector.tensor_add(stats_bn[:, 0:4], stats_bn[:, 0:4],
                                 sbn_p[:, 12:16])
            for co in range(CO):
                scr = p_scr.tile([128, T_LOC], bf16, tag="scr")
                nc.scalar.activation(scr[:], wy_t[:, co, :], SQUARE,
                                     accum_out=stats_bn[:, 4 + co:5 + co])
            for fc in range(4):
                for co in range(CO):
                    ps = ps_c.tile([128, 512], f32, tag="c")
                    for ci in range(CI):
                        nc.tensor.matmul(
                            ps[:, :FC],
                            wt_m[:, ci, co * 128:(co + 1) * 128],
                            xloc_t[:, ci, fc * FC:(fc + 1) * FC],
                            start=(ci == 0), stop=(ci == CI - 1))
                    nc.vector.tensor_scalar_add(
                        pm_t[:, co, fc * FC:(fc + 1) * FC],
                        ps[:, :FC], bm_t[:, co:co + 1])
            p_scr.release()
            if debug:
                dwy_p = tc.alloc_tile_pool(name="dwyp", bufs=1)
                dwy_f = dwy_p.tile([128, CO, T_LOC], f32, tag="dwy")
                for co in range(CO):
                    nc.vector.tensor_copy(dwy_f[:, co, :], wy_t[:, co, :])
                nc.sync.dma_start(out=dview(d_wy), in_=dwy_f[:])
                dwy_p.release()
            p_xl.release()
            p_w2.release()

            # ======== P1: K/V convs over full s-range (bf16 outputs) ========
            p_kv = tc.alloc_tile_pool(name="kvp", bufs=1, side="right")
            pgh = p_kv.tile([128, CI, S_PAD], fp8, tag="pgh")
            phmh = p_kv.tile([128, NST, C], bf16, tag="phmh")
            p_piece = tc.alloc_tile_pool(name="piecep", bufs=2)

            pieces = []
            o = 0
            while o < NST:
                w = min(4, NST - o)
                pieces.append((o, w))
                o += w
            for (pt0, ptw) in pieces:
                s_off = pt0 * 128
                pw = ptw * 128
                xp = p_piece.tile([128, CI, 512], fp8, tag="xp", bufs=3,
                                  name="xp")
                nc.sync.dma_start(
                    out=xp[:, :, :pw],
                    in_=dview(x8_full)[:, :, s_off:s_off + pw])
                for co in range(CO):
                    ps = ps_c.tile([128, 512], f32, tag="c")
                    for k in range(2):
                        nc.tensor.matmul(
                            ps[:, :pw],
                            wt_g8[:, 2 * k:2 * k + 2, co * 128:(co + 1) * 128],
                            xp[:, 2 * k:2 * k + 2, :pw],
                            start=(k == 0), stop=(k == 1),
                            perf_mode=DBLROW)
                    nc.vector.tensor_scalar_add(
                        pgh[:, co, s_off:s_off + pw],
                        ps[:, :pw], bg_t[:, co:co + 1])
                mp = p_piece.tile([128, CI, 512], fp8, tag="mp", bufs=3,
                                  name="mp")
                nc.gpsimd.dma_start(
                    out=mp[:, :, :pw],
                    in_=dview(mask8_full)[:, :, s_off:s_off + pw])
                for sj in range(ptw):
                    st = pt0 + sj
                    ps = ps_c.tile([128, 512], f32, tag="c")
                    for k in range(2):
                        nc.tensor.matmul(
                            ps[:],
                            mp[:, 2 * k:2 * k + 2, sj * 128:(sj + 1) * 128],
                            wt_h8[:, 2 * k:2 * k + 2, :],
                            start=(k == 0), stop=(k == 1),
                            perf_mode=DBLROW)
                    nc.scalar.activation(phmh[:, st, :], ps[:], COPY_FN)

            if debug:
                p_dbg = tc.alloc_tile_pool(name="dbgp", bufs=1)
                dphx_f = p_dbg.tile([128, CI, T_LOC], f32, tag="dphx")
                for ci in range(CI):
                    nc.vector.tensor_copy(dphx_f[:, ci, :], phx[:, ci, :])
                nc.sync.dma_start(out=dview(d_phx), in_=dphx_f[:])
                p_dbg.release()
            ps_c.release()
            p_piece.release()
            p_w1.release()

            # ======== P2: attention, single pass, [c,t]-form PV ========
            # PSUM: o x4 (c-chunks) + e x3 (rb shares the e tag) + r = 8 banks
            ps_att = tc.alloc_tile_pool(name="psa", bufs=1, space="PSUM")
            p_z = tc.alloc_tile_pool(name="zp", bufs=2)
            for bi, (t0, nt) in enumerate(BLOCKS):
                tfree = nt * 128
                trange = slice(t0 * 128, t0 * 128 + tfree)
                ops = [ps_att.tile([128, 512], f32, tag=f"o{j}", name=f"o{j}")
                       for j in range(CO)]
                rps = ps_att.tile([128, 512], f32, tag="r", name="rps")
                for st in range(NST):
                    eps_t = ps_att.tile([128, 512], f32, tag="e", bufs=3,
                                        name="eps")
                    for k in range(2):
                        nc.tensor.matmul(
                            eps_t[:, :tfree],
                            pgh[:, 2 * k:2 * k + 2, st * 128:(st + 1) * 128],
                            phx[:, 2 * k:2 * k + 2, trange],
                            start=(k == 0), stop=(k == 1),
                            perf_mode=DBLROW)
                    pt = ptpool.tile([128, 512], bf16, tag="pt")
                    nc.scalar.activation(pt[:, :tfree], eps_t[:, :tfree],
                                         EXP, bias=m1b[:], scale=1.0)
                    for co in range(CO):
                        nc.tensor.matmul(
                            ops[co][:, :tfree],
                            phmh[:, st, co * 128:(co + 1) * 128],
                            pt[:, :tfree],
                            start=(st == 0), stop=(st == NST - 1))
                    nc.tensor.matmul(
                        rps[:, :tfree],
                        ones_t[:],
                        pt[:, :tfree],
                        start=(st == 0), stop=(st == NST - 1))

                # block tail: 1/r broadcast, z = psum*rb, expz, se partials,
                # mt0 = expz*pm folded into pm_t in place
                nc.vector.tensor_add(racc_row[0:1, trange], rps[0:1, :tfree],
                                     maskhuge[0:1, trange])
                rb_ps = ps_att.tile([128, 512], f32, tag="e", bufs=3,
                                     name="rbps")
                nc.tensor.matmul(rb_ps[:, :tfree], ones_row[:],
                                 racc_row[0:1, trange],
                                 start=True, stop=True)
                rb = p_z.tile([128, 512], f32, tag="rb")
                nc.vector.reciprocal(rb[:, :tfree], rb_ps[:, :tfree])
                se_blk = spool.tile([128, CO], f32, tag=f"seblk{bi}")
                for co in range(CO):
                    zt = p_z.tile([128, 512], f32, tag="z")
                    nc.vector.tensor_mul(zt[:, :tfree], ops[co][:, :tfree],
                                         rb[:, :tfree])
                    ez = p_z.tile([128, 512], bf16, tag="ez")
                    nc.scalar.activation(ez[:, :tfree], zt[:, :tfree],
                                         EXP, bias=m2bh[:, co:co + 1], scale=1.0,
                                         accum_out=se_blk[:, co:co + 1])
                    nc.vector.tensor_mul(pm_t[:, co, trange], ez[:, :tfree],
                                         pm_t[:, co, trange])
                    if debug:
                        nc.sync.dma_start(out=dview(d_z)[:, co, trange],
                                          in_=zt[:, :tfree])
                if bi == 0:
                    nc.vector.tensor_copy(se_tot[:], se_blk[:])
                    # BN collective gated on block-0 completion: it then runs
                    # in a DMA-quiet window instead of starving the piece
                    # stream (its rings monopolize the DMA engines). The
                    # rewrite of stats_bn[:, 0:1] below adds the dependency.
                    one_dep = spool.tile([128, 1], f32, tag="onedep")
                    nc.vector.tensor_scalar(one_dep[:], se_blk[:, 0:1],
                                            0.0, 1.0, op0=MUL, op1=ADD)
                    nc.vector.tensor_scalar_mul(stats_bn[:, 0:1],
                                                stats_bn[:, 0:1], one_dep[:])
                    nc.gpsimd.dma_start(out=cc_bn_in[:], in_=stats_bn[:])
                    nc.gpsimd.collective_compute(
                        "AllReduce", mybir.AluOpType.add,
                        replica_groups=[[0, 1, 2, 3, 4, 5, 6, 7]],
                        ins=[cc_bn_in[:]], outs=[cc_bn_out[:]])
                else:
                    nc.vector.tensor_add(se_tot[:], se_tot[:], se_blk[:])
            ps_att.release()
            p_z.release()
            p_kv.release()
            p_phx.release()

            # ======== P3: SE collective + finale ========
            stats_se = spool.tile([128, 8], f32, tag="statsse")
            nc.vector.tensor_scalar_mul(stats_se[:, 0:CO], se_tot[:],
                                        bsel_t[:, 0:1])
            nc.vector.tensor_scalar_mul(stats_se[:, CO:2 * CO], se_tot[:],
                                        bsel_t[:, 1:2])
            nc.sync.dma_start(out=cc_se_in[:], in_=stats_se[:])
            nc.gpsimd.collective_compute(
                "AllReduce", mybir.AluOpType.add,
                replica_groups=[[0, 1, 2, 3, 4, 5, 6, 7]],
                ins=[cc_se_in[:]], outs=[cc_se_out[:]])

            # BN finale scales: depend only on the EARLY collective, so they
            # run under the SE collective
            p_ot = tc.alloc_tile_pool(name="otp", bufs=1)
            ot_t = p_ot.tile([128, CO, T_LOC], f32, tag="ot")
            gbn = spool.tile([128, 8], f32, tag="gbn")
            nc.gpsimd.dma_start(out=gbn[:], in_=cc_bn_out[:])
            cnt = 1.0 / (N_B * THW)
            mu = spool.tile([128, CO], f32, tag="mu")
            nc.vector.tensor_scalar_mul(mu[:], gbn[:, 0:CO], cnt)
            nc.vector.tensor_sub(mu[:], mu[:], bzc_t[:, 0:CO])
            ex2 = spool.tile([128, CO], f32, tag="ex2")
            nc.vector.tensor_scalar_mul(ex2[:], gbn[:, 4:4 + CO], cnt)
            nc.vector.tensor_sub(ex2[:], ex2[:], bzc_t[:, CO:2 * CO])
            var = spool.tile([128, CO], f32, tag="var")
            nc.vector.tensor_mul(var[:], mu[:], mu[:])
            nc.vector.tensor_sub(var[:], ex2[:], var[:])
            nc.vector.tensor_scalar_add(var[:], var[:], BN_EPS)
            std = spool.tile([128, CO], f32, tag="std")
            nc.scalar.activation(std[:], var[:], SQRT)
            nc.vector.reciprocal(std[:], std[:])
            alpha = spool.tile([128, CO], f32, tag="alpha")
            nc.vector.tensor_mul(alpha[:], std[:], bnw_t[:])
            beta = spool.tile([128, CO], f32, tag="beta")
            nc.vector.tensor_mul(beta[:], mu[:], alpha[:])
            nc.vector.tensor_sub(beta[:], bnb_t[:], beta[:])
            for co in range(CO):
                nc.vector.tensor_scalar(ot_t[:, co, :], wy_t[:, co, :],
                                        alpha[:, co:co + 1], beta[:, co:co + 1],
                                        op0=MUL, op1=ADD)

            gst = spool.tile([128, 8], f32, tag="gst")
            nc.sync.dma_start(out=gst[:], in_=cc_se_out[:])
            gse = spool.tile([128, CO], f32, tag="gse")
            tmp_a = spool.tile([128, CO], f32, tag="tmpa")
            nc.vector.tensor_scalar_mul(gse[:], gst[:, 0:CO], bsel_t[:, 0:1])
            nc.vector.tensor_scalar_mul(tmp_a[:], gst[:, CO:2 * CO],
                                        bsel_t[:, 1:2])
            nc.vector.tensor_add(gse[:], gse[:], tmp_a[:])
            nc.vector.reciprocal(gse[:], gse[:])
            nc.vector.tensor_scalar_mul(gse[:], gse[:], gamma)

            p_out = tc.alloc_tile_pool(name="outp", bufs=4)
            for co in range(CO):
                mt = p_out.tile([128, T_LOC], f32, tag="mt")
                nc.scalar.activation(mt[:], pm_t[:, co, :], COPY_FN,
                                     scale=gse[:, co:co + 1])
                nc.vector.tensor_add(mt[:], mt[:], ot_t[:, co, :])
                nc.sync.dma_start(out=dview(out_loc)[:, co, :], in_=mt[:])
            p_out.release()
            p_ot.release()
            p_res.release()

    nc.compile()
    return nc


def _prepare_maps(x, mask, Wh, bh, Wg, bg, Wm, bm, Wz, bz, bn_w, bn_b):
    import ml_dtypes

    xf = np.ascontiguousarray(x.reshape(N_B, C, THW), dtype=np.float32)
    mf = np.ascontiguousarray(mask.reshape(N_B, C, THW), dtype=np.float32)

    def chunked_bias(b):
        return np.ascontiguousarray(b.reshape(CO, 128).T, dtype=np.float32)

    wht = np.ascontiguousarray(Wh.T, dtype=np.float32)
    wgt = np.ascontiguousarray(Wg.T, dtype=np.float32)
    wmt = np.ascontiguousarray(Wm.T, dtype=np.float32)
    wzt = np.ascontiguousarray(Wz.T, dtype=np.float32)
    wht8 = wht.astype(ml_dtypes.float8_e4m3)
    wgt8 = wgt.astype(ml_dtypes.float8_e4m3)
    bh_row = np.broadcast_to(bh.astype(np.float32), (128, C)).copy()

    # BN bias compensation: raw sums include (8*T_LOC - N*THW) padded columns
    # where wy == bz exactly (x padded with zeros).
    n_pad = 8 * T_LOC - N_B * THW
    cntf = 1.0 / (N_B * THW)
    bzc = np.zeros((128, 8), np.float32)
    bzc[:, 0:4] = chunked_bias(bz * (n_pad * cntf))
    bzc[:, 4:8] = chunked_bias((bz * bz) * (n_pad * cntf))

    in_maps = []
    for core in range(8):
        n, q = divmod(core, 4)
        t0 = T_LOC * q
        valid = int(np.clip(THW - t0, 0, T_LOC))
        x_locc = np.zeros((C, T_LOC), np.float32)
        x_locc[:, :valid] = xf[n][:, t0:t0 + valid]
        x8_locc = x_locc.astype(ml_dtypes.float8_e4m3)
        # additive +huge mask on r for invalid t: 1/(r+1e30) ~ 0 => z ~ 0
        # => expz ~ exp(-M2) ~ 0
        maskhuge = np.full((1, T_LOC), 1e-30, np.float32)
        maskhuge[0, valid:] = 1e30
        bsel = np.zeros((128, 2), np.float32)
        bsel[:, 0] = 1.0 if n == 0 else 0.0
        bsel[:, 1] = 0.0 if n == 0 else 1.0
        in_maps.append(dict(
            x8_full=xf[n].astype(ml_dtypes.float8_e4m3),
            mask8_full=mf[n].astype(ml_dtypes.float8_e4m3),
            x_loc=x_locc, x8_loc=x8_locc, wht8=wht8, wgt8=wgt8,
            wmt=wmt, wzt=wzt,
            bh_in=chunked_bias(bh), bg_in=chunked_bias(bg),
            bm_in=chunked_bias(bm), bz_in=chunked_bias(bz),
            bh_row_in=bh_row,
            bnw_in=chunked_bias(bn_w), bnb_in=chunked_bias(bn_b),
            ones_in=np.ones((128, 128), dtype=ml_dtypes.bfloat16),
            ones_row_in=np.ones((1, 128), np.float32),
            maskhuge_in=maskhuge, bzc_in=bzc,
            bsel_in=bsel,
        ))
    return in_maps


def _estimate_shifts(xf, mf, Wh, bh, Wg, bg):
    # M1: safe global upper-bound estimate for the max of the energy matrix.
    # Any M1 in [true_max - 80, min_row_max + 85] keeps softmax exact
    # (constant shifts cancel); the window is tens wide so a sampled
    # estimate plus margin is bulletproof.
    ti = np.arange(0, THW, 41)
    si = np.arange(0, THW, 7)
    m_s = -np.inf
    for n in range(N_B):
        Q = (Wh @ xf[n][:, ti]) + bh[:, None]
        K = (Wg @ xf[n][:, si]) + bg[:, None]
        m_s = max(m_s, float((Q.T @ K).max()))
    m1 = m_s + 5.0
    # M2: norm bound on |ph_m| entries (second softmax argument is a convex
    # combination of ph_m values, so bounded by max |ph_m|).
    whn = float(np.linalg.norm(Wh, axis=1).max())
    mcn = max(float(np.linalg.norm(mf[n], axis=0).max()) for n in range(N_B))
    m2 = whn * mcn + float(np.abs(bh).max()) + 1.0
    return m1, m2


def kernel(x, mask, Wh, bh, Wg, bg, Wm, bm, Wz, bz, bn_w, bn_b, gamma,
           _debug=False, _trace=False):
    from concourse.bass_utils import run_bass_kernel_spmd

    x = np.asarray(x, np.float32)
    mask = np.asarray(mask, np.float32)
    Wh = np.asarray(Wh, np.float32); bh = np.asarray(bh, np.float32)
    Wg = np.asarray(Wg, np.float32); bg = np.asarray(bg, np.float32)
    Wm = np.asarray(Wm, np.float32); bm = np.asarray(bm, np.float32)
    Wz = np.asarray(Wz, np.float32); bz = np.asarray(bz, np.float32)
    bn_w = np.asarray(bn_w, np.float32); bn_b = np.asarray(bn_b, np.float32)
    gammaf = float(np.asarray(gamma))

    xf = x.reshape(N_B, C, THW)
    mf = mask.reshape(N_B, C, THW)
    m1, m2 = _estimate_shifts(xf, mf, Wh, bh, Wg, bg)
    key = (round(m1, 1), round(m2, 1), round(gammaf, 6), bool(_debug))
    if key not in _PROG_CACHE:
        _PROG_CACHE[key] = _build_program(key[0], key[1], gammaf, debug=_debug)
    nc = _PROG_CACHE[key]

    in_maps = _prepare_maps(x, mask, Wh, bh, Wg, bg, Wm, bm, Wz, bz, bn_w, bn_b)
    res = run_bass_kernel_spmd(nc, in_maps, core_ids=list(range(8)), trace=_trace)

    out = np.empty((N_B, C, THW), np.float32)
    for core in range(8):
        n, q = divmod(core, 4)
        t0 = T_LOC * q
        valid = int(np.clip(THW - t0, 0, T_LOC))
        if valid > 0:
            out[n][:, t0:t0 + valid] = res.results[core]["out_loc"][:, :valid]
    out = out.reshape(N_B, C, T, H, W)
    if _debug or _trace:
        return out, res
    return out
